# revision 2
# baseline (speedup 1.0000x reference)
"""CNF vector field + exact divergence kernel for Trainium2 (8 NeuronCores).

Math (per sample x of dim D=64, t scalar, 3-layer MLP 65->512->512->64):
    h1 = tanh(W1^T [t;x] + b1)
    h2 = tanh(W2^T h1 + b2)
    dx = W3^T h2 + b3
    div = trace(d dx / d x) collapses to the bilinear form
        div = (1-h1^2)^T G (1-h2^2) = (h1^2-1)^T G (h2^2-1)
    with G = W2 * (W1[1:].T @ W3.T)   (elementwise product, [512,512])

Sharding: pure data parallel over batch; 8192/8 = 1024 samples per core;
weights replicated.  All activations are kept feature-major ([feature
partitions, batch free]) so every matmul chains with weights stationary and
no activation transposes; only the input x (8 PE transposes) and the final
output (8 PE transposes) cross layouts.
"""

import sys

if "/opt/trn_rl_repo" not in sys.path:
    sys.path.insert(0, "/opt/trn_rl_repo")

import numpy as np

D = 64
H = 512
B = 8192
N_CORES = 8
BC = B // N_CORES          # 1024 samples per core
NCH = 2                    # batch chunks per core (fp32 moving operand max 512)
CH = BC // NCH             # 512
KT = H // 128              # 4 k-tiles of the hidden dim
BT = BC // 128             # 8 batch tiles of 128 (for transposes)

_CACHE = {}


def _patch_tile_drain():
    """walrus in this toolchain accepts only one sync wait per CTRL
    instruction; split the TileContext tail-drain waits across nops."""
    import concourse.mybir as mybir
    from concourse.tile import TileContext
    from concourse.vector_clock import ScopedClock

    if getattr(TileContext, "_drain_patched", False):
        return

    def _drain_and_barrier(self, tick_clock, wait_clock):
        nc = self.nc
        probe = nc.sync.nop(nofuse=True, hint="drain_wait_probe")
        wait_clock.add_sem_waits(
            probe.ins, ScopedClock({None: tick_clock.global_clock})
        )
        waits = list(probe.ins.sync_info.on_wait) if probe.ins.sync_info else []
        if len(waits) > 1:
            probe.ins.sync_info.on_wait.clear()
            probe.ins.sync_info.on_wait.append(waits[0])
            for w in waits[1:]:
                nop_inst = nc.sync.nop(nofuse=True, hint="drain_wait_split")
                if nop_inst.ins.sync_info is None:
                    nop_inst.ins.sync_info = mybir.SyncInfo(on_wait=[], on_update=[])
                nop_inst.ins.sync_info.on_wait.append(w)
        nc.sync.drain()  # SP already observed every sem above
        nc.all_engine_barrier()
        popped = nc._tile_sem_poison_stack.pop()
        assert popped is self._sem_poison
        # sem clears skipped: NRT reloads sem state per execution; verified
        # by repeated-call correctness checks in test.py

    TileContext._orig_drain_and_barrier = TileContext._drain_and_barrier
    TileContext._drain_and_barrier = _drain_and_barrier
    TileContext._drain_patched = True


# this walrus build has small per-instruction sync-wait budgets; split any
# excess waits onto same-engine nops placed just before the instruction
# (waiting earlier on the same engine stream is always safe).
_WAIT_LIMITS = {"DMACOPY": 1, "NOOP": 1, "DRAIN": 1, "TRIGGEREDCOPY": 1}
_DEFAULT_WAIT_LIMIT = 1


def _split_excess_waits(nc):
    import concourse.mybir as mybir

    ctr = 0
    for fn in nc.m.functions:
        for blk in fn.blocks:
            lst = blk.instructions
            out = []
            changed = False
            for inst in lst:
                si = inst.sync_info
                waits = list(si.on_wait) if si else []
                opname = type(inst).__name__.replace("Inst", "").upper()
                limit = _WAIT_LIMITS.get(opname, _DEFAULT_WAIT_LIMIT)
                if len(waits) > limit:
                    keep = waits[-limit:]
                    excess = waits[:-limit]
                    si.on_wait.clear()
                    for w in keep:
                        si.on_wait.append(w)
                    for w in excess:
                        nop = mybir.InstNoOp(name=f"WSPLIT-{ctr}", ins=[], outs=[])
                        ctr += 1
                        nop.engine = inst.engine
                        nop.sync_info = mybir.SyncInfo(on_wait=[w], on_update=[])
                        out.append(nop)
                    changed = True
                out.append(inst)
            if changed:
                lst[:] = out


def _build(mm_dtype_name="float32r", for_sim=False):
    import concourse.bass as bass
    import concourse.mybir as mybir
    from concourse.masks import make_identity
    from concourse.tile import TileContext

    _patch_tile_drain()

    f32 = mybir.dt.float32
    mmdt = getattr(mybir.dt, mm_dtype_name)
    AF = mybir.ActivationFunctionType
    OP = mybir.AluOpType

    def mm(ap):
        return ap.bitcast(mmdt) if mmdt != f32 else ap

    nc = bass.Bass(trn_type="TRN2")
    nc._bass_sim_build = for_sim

    x_s = nc.dram_tensor("x_s", [BC, D + 2], f32, kind="ExternalInput")
    w1 = nc.dram_tensor("w1", [D + 1, H], f32, kind="ExternalInput")
    b1 = nc.dram_tensor("b1", [H], f32, kind="ExternalInput")
    w2 = nc.dram_tensor("w2", [H, H], f32, kind="ExternalInput")
    b2 = nc.dram_tensor("b2", [H], f32, kind="ExternalInput")
    w3 = nc.dram_tensor("w3", [H, D], f32, kind="ExternalInput")
    b3 = nc.dram_tensor("b3", [D], f32, kind="ExternalInput")
    t_in = nc.dram_tensor("t_in", [1], f32, kind="ExternalInput")
    out = nc.dram_tensor("out", [BC, D + 1], f32, kind="ExternalOutput")

    with TileContext(nc) as tc:
        with (
            tc.tile_pool(name="weights", bufs=1) as wpool,
            tc.tile_pool(name="acts", bufs=1) as apool,
            tc.tile_pool(name="xin", bufs=1) as xpool,
            tc.tile_pool(name="psmm", bufs=6, space="PSUM") as psmm,
            tc.tile_pool(name="pstr", bufs=2, space="PSUM") as pstr,
            tc.tile_pool(name="outbuf", bufs=1) as opool,
        ):
            # identity first: gpsimd must build it before its SWDGE DMAs
            ident = wpool.tile([128, 128], f32)
            make_identity(nc, ident)
            identr = wpool.tile([128, 128], mmdt)
            nc.vector.tensor_copy(identr, ident)
            ones_f32 = wpool.tile([128, 1], f32)
            nc.vector.memset(ones_f32, 1.0)
            ones_col = wpool.tile([128, 1], mmdt)
            nc.vector.tensor_copy(ones_col, ones_f32)

            # ---------------- input / weight DMAs --------------------------
            # queues: SP = x half0, W1x, W3, W2; ACT = t, x half1;
            # gpsimd SWDGE = small bias rows/gathers
            x_sb = xpool.tile([128, BT, D + 2], mmdt)
            half = BT // 2
            nc.sync.dma_start(
                out=x_sb[:, 0:half, :],
                in_=x_s[0 : half * 128]
                .rearrange("(bt p) c -> p bt c", p=128)
                .bitcast(mmdt),
            )
            nc.scalar.dma_start(
                out=x_sb[:, half:BT, :],
                in_=x_s[half * 128 : BC]
                .rearrange("(bt p) c -> p bt c", p=128)
                .bitcast(mmdt),
            )

            # W1hat rows: 0-63 = W1[1:65], 64 = W1[0,:] (t injected via
            # xhat's t-row), 65 = b1
            w1hat = wpool.tile([D + 2, H], mmdt)
            nc.sync.dma_start(out=w1hat[0:D, :], in_=w1[1 : D + 1, :].bitcast(mmdt))
            nc.gpsimd.dma_start(
                out=w1hat[D : D + 1, :], in_=w1[0:1, :].bitcast(mmdt)
            )
            nc.gpsimd.dma_start(
                out=w1hat[D + 1 : D + 2, :], in_=b1[None, :].bitcast(mmdt)
            )

            w34 = wpool.tile([128, KT, D], mmdt)
            nc.sync.dma_start(
                out=w34, in_=w3[:].rearrange("(t p) m -> p t m", p=128).bitcast(mmdt)
            )
            w24 = wpool.tile([128, KT, H], mmdt)
            nc.sync.dma_start(
                out=w24, in_=w2[:].rearrange("(t p) m -> p t m", p=128).bitcast(mmdt)
            )
            b2t = wpool.tile([128, KT], f32)
            nc.gpsimd.dma_start(out=b2t, in_=b2[:].rearrange("(t p) -> p t", p=128))
            b3t = wpool.tile([D, 1], f32)
            nc.gpsimd.dma_start(out=b3t, in_=b3[:, None])

            # ---------------- xhatT assembly (per-chunk tiles) -------------
            # x_s columns 64/65 hold t and 1 (host-prepped), so each PE
            # transpose emits complete xhat rows [x^T; t*ones; ones]
            xhat = []
            for n in range(NCH):
                xh = apool.tile([D + 2, CH], mmdt, tag=f"xhat{n}", name=f"xhat{n}")
                xhat.append(xh)
                pti = pstr.tile([D + 2, 512], mmdt, tag="tr", name=f"pti{n}")
                for j in range(4):
                    bt = n * 4 + j
                    nc.tensor.transpose(
                        pti[:, j * 128 : (j + 1) * 128],
                        x_sb[:, bt, :],
                        identr,
                    )
                nc.vector.tensor_copy(xh, pti)

            # ---------------- L1: h1 = tanh(W1hat^T @ xhat) ----------------
            h1 = [apool.tile([128, KT, CH], mmdt, tag=f"h1_{n}", name=f"h1_{n}") for n in range(NCH)]
            u1 = [apool.tile([128, KT, CH], f32, tag=f"u1_{n}", name=f"u1_{n}") for n in range(NCH)]
            s1m = [apool.tile([128, KT, CH], mmdt, tag=f"s1m{n}", name=f"s1m{n}") for n in range(NCH)]
            for n in range(NCH):
                for i in range(KT):
                    pz = psmm.tile([128, CH], f32, tag="mmtile")
                    nc.tensor.matmul(
                        pz,
                        mm(w1hat[:, i * 128 : (i + 1) * 128]),
                        mm(xhat[n]),
                        start=True,
                        stop=True,
                    )
                    nc.scalar.activation(h1[n][:, i, :], pz, AF.Tanh)
                    nc.gpsimd.tensor_scalar_add(u1[n][:, i, :], h1[n][:, i, :], 1.0)

            # ---------------- W3T + G = W2 * (W1x^T @ W3^T) ----------------
            w3t = apool.tile([D, H], mmdt)
            ptw = pstr.tile([D, 512], mmdt, tag="tr")
            for k in range(KT):
                nc.tensor.transpose(
                    ptw[:, k * 128 : (k + 1) * 128], w34[:, k, :], identr
                )
            nc.vector.tensor_copy(w3t, ptw)

            g4 = apool.tile([128, KT, H], mmdt)
            for i in range(KT):
                pg = psmm.tile([128, H], f32, tag="mmtile")
                nc.tensor.matmul(
                    pg,
                    mm(w1hat[0:D, i * 128 : (i + 1) * 128]),
                    mm(w3t),
                    start=True,
                    stop=True,
                )
                nc.vector.tensor_mul(g4[:, i, :], w24[:, i, :], pg)

            # s1m after g4 muls so DVE unblocks the G path first
            for n in range(NCH):
                nc.vector.scalar_tensor_tensor(
                    out=s1m[n],
                    in0=h1[n],
                    scalar=1.0,
                    in1=u1[n],
                    op0=OP.subtract,
                    op1=OP.mult,
                )

            # ---------------- L2: h2 = tanh(W2^T h1 + b2) ------------------
            h2 = [apool.tile([128, KT, CH], mmdt, tag=f"h2_{n}", name=f"h2_{n}") for n in range(NCH)]
            s2q = [apool.tile([128, KT, CH], f32, tag=f"s2q{n}", name=f"s2q{n}") for n in range(NCH)]
            for n in range(NCH):
                for i in range(KT):
                    pz = psmm.tile([128, CH], f32, tag="mmtile")
                    for k in range(KT):
                        nc.tensor.matmul(
                            pz,
                            mm(w24[:, k, i * 128 : (i + 1) * 128]),
                            mm(h1[n][:, k, :]),
                            start=(k == 0),
                            stop=(k == KT - 1),
                        )
                    nc.scalar.activation(
                        h2[n][:, i, :], pz, AF.Tanh, bias=b2t[:, i : i + 1]
                    )
                    nc.gpsimd.tensor_mul(
                        s2q[n][:, i, :], h2[n][:, i, :], h2[n][:, i, :]
                    )

            # ---------------- c = G^T s1m ; e = (s2q - 1) * c --------------
            ebuf = [apool.tile([128, KT, CH], mmdt, tag=f"eb{n}", name=f"eb{n}") for n in range(NCH)]
            for n in range(NCH):
                for i in range(KT):
                    pc = psmm.tile([128, CH], f32, tag="mmtile")
                    for k in range(KT):
                        nc.tensor.matmul(
                            pc,
                            mm(g4[:, k, i * 128 : (i + 1) * 128]),
                            mm(s1m[n][:, k, :]),
                            start=(k == 0),
                            stop=(k == KT - 1),
                        )
                    nc.vector.scalar_tensor_tensor(
                        out=ebuf[n][:, i, :],
                        in0=s2q[n][:, i, :],
                        scalar=1.0,
                        in1=pc,
                        op0=OP.subtract,
                        op1=OP.mult,
                    )

            # ---------------- L3 + div + output ----------------------------
            ob = opool.tile([128, BT, D + 1], f32)
            for n in range(NCH):
                outT = apool.tile([D, CH], f32, tag=f"outT{n}", name=f"outT{n}")
                dv_sb = apool.tile([1, CH], f32, tag=f"dv{n}", name=f"dv{n}")
                pd = psmm.tile([1, CH], f32, tag="mmtile", name=f"pd{n}")
                for k in range(KT):
                    nc.tensor.matmul(
                        pd,
                        mm(ones_col),
                        mm(ebuf[n][:, k, :]),
                        start=(k == 0),
                        stop=(k == KT - 1),
                    )
                nc.scalar.activation(dv_sb, pd, AF.Identity)
                po = psmm.tile([D, CH], f32, tag="mmtile", name=f"po{n}")
                for k in range(KT):
                    nc.tensor.matmul(
                        po,
                        mm(w34[:, k, :]),
                        mm(h2[n][:, k, :]),
                        start=(k == 0),
                        stop=(k == KT - 1),
                    )
                nc.scalar.activation(outT, po, AF.Identity, bias=b3t[:, 0:1])

                pto = pstr.tile([128, 4, D + 1], f32, tag="tr", name=f"pto{n}")
                for j in range(4):
                    nc.tensor.transpose(
                        pto[:, j, 0:D],
                        outT[:, j * 128 : (j + 1) * 128],
                        ident[0:D, 0:D],
                    )
                    nc.tensor.transpose(
                        pto[:, j, D : D + 1],
                        dv_sb[:, j * 128 : (j + 1) * 128],
                        ident[0:1, 0:1],
                    )
                nc.vector.tensor_copy(ob[:, n * 4 : (n + 1) * 4, :], pto)
                nc.sync.dma_start(
                    out=out[n * CH : (n + 1) * CH, :].rearrange(
                        "(bt p) c -> p bt c", p=128
                    ),
                    in_=ob[:, n * 4 : (n + 1) * 4, :],
                )

    if not for_sim:
        _split_excess_waits(nc)
    return nc


def _get_nc():
    if "nc" not in _CACHE:
        _CACHE["nc"] = _build()
    return _CACHE["nc"]


def _make_in_maps(inputs):
    t = np.asarray(inputs["t"], np.float32)
    x = np.asarray(inputs["x"], np.float32)
    # columns 64/65 carry t and 1 so the device transposes produce the
    # [x^T; t*ones; ones] contraction operand directly
    x_aug = np.empty((B, D + 2), np.float32)
    x_aug[:, 0:D] = x[:, 0:D]
    x_aug[:, D] = t[0]
    x_aug[:, D + 1] = 1.0
    base = {
        "w1": np.ascontiguousarray(np.asarray(inputs["W1"], np.float32)),
        "b1": np.ascontiguousarray(np.asarray(inputs["b1"], np.float32)),
        "w2": np.ascontiguousarray(np.asarray(inputs["W2"], np.float32)),
        "b2": np.ascontiguousarray(np.asarray(inputs["b2"], np.float32)),
        "w3": np.ascontiguousarray(np.asarray(inputs["W3"], np.float32)),
        "b3": np.ascontiguousarray(np.asarray(inputs["b3"], np.float32)),
        "t_in": t,
    }
    return [
        dict(base, x_s=np.ascontiguousarray(x_aug[i * BC : (i + 1) * BC]))
        for i in range(N_CORES)
    ]


def _gather(res):
    return np.concatenate([res.results[i]["out"] for i in range(N_CORES)], axis=0)


def kernel(t, x, W1, b1, W2, b2, W3, b3):
    from concourse.bass_utils import run_bass_kernel_spmd

    nc = _get_nc()
    in_maps = _make_in_maps(
        dict(t=t, x=x, W1=W1, b1=b1, W2=W2, b2=b2, W3=W3, b3=b3)
    )
    res = run_bass_kernel_spmd(nc, in_maps, core_ids=list(range(N_CORES)))
    _CACHE["last_result"] = res
    out = _gather(res)
    # flaky-core guard: a dropped execution leaves the donated zero output
    # buffer untouched; the true output of this MLP is never all-zero.
    for _ in range(3):
        bad = [
            i
            for i in range(N_CORES)
            if not np.any(res.results[i]["out"][:, 0:D])
        ]
        if not bad:
            break
        res = run_bass_kernel_spmd(nc, in_maps, core_ids=list(range(N_CORES)))
        _CACHE["last_result"] = res
        out = _gather(res)
    return out



# revision 5
# speedup vs baseline: 1.6210x; 1.6210x over previous
"""CNF vector field + exact divergence kernel for Trainium2 (8 NeuronCores).

Math (per sample x of dim D=64, t scalar, 3-layer MLP 65->512->512->64):
    h1 = tanh(W1^T [x;t;1])          (w1hat rows: W1[1:], W1[0], b1)
    h2 = tanh(W2^T h1 + b2)
    dx = W3^T h2 + b3
    div = trace(d dx / d x) = (h1^2-1)^T G (h2^2-1)
    with G = W2 * (W1[1:].T @ W3.T)   (elementwise product, [512,512])
G is computed on HOST (weights-only, 33 MFLOP numpy) and DMA'd.

Sharding: pure data parallel over batch; 8192/8 = 1024 samples per core;
weights replicated.  Activations stay feature-major ([feature partitions,
batch free]); only the input x (4 PE transposes per chunk) and the final
output (4 PE transposes per chunk) cross layouts.

Engine budget per core (target ~19us):  PE ~44k cycles of matmul striping
(L1 8, L2 32, G^T m1 32, L3 8, div 2, transposes 16 instrs), ACT 20
activations (tanh x16 + bias-adds), DVE squares/sub/e-products/tree-adds.
gpsimd does NOTHING (measured 7.5us per elementwise op in v1 = 76us busy).
"""

import sys

if "/opt/trn_rl_repo" not in sys.path:
    sys.path.insert(0, "/opt/trn_rl_repo")

import numpy as np

D = 64
H = 512
B = 8192
N_CORES = 8
BC = B // N_CORES          # 1024 samples per core
NCH = 2                    # batch chunks per core (fp32 moving operand max 512)
CH = BC // NCH             # 512
KT = H // 128              # 4 k-tiles of the hidden dim
BT = BC // 128             # 8 batch tiles of 128 (for transposes)

MM_DTYPE = "float32r"      # or "bfloat16"

_CACHE = {}


def _patch_tile_drain():
    """walrus in this toolchain accepts only one sync wait per CTRL
    instruction; split the TileContext tail-drain waits across nops."""
    import concourse.mybir as mybir
    from concourse.tile import TileContext
    from concourse.vector_clock import ScopedClock

    if getattr(TileContext, "_drain_patched", False):
        return

    def _drain_and_barrier(self, tick_clock, wait_clock):
        nc = self.nc
        probe = nc.sync.nop(nofuse=True, hint="drain_wait_probe")
        wait_clock.add_sem_waits(
            probe.ins, ScopedClock({None: tick_clock.global_clock})
        )
        waits = list(probe.ins.sync_info.on_wait) if probe.ins.sync_info else []
        if len(waits) > 1:
            probe.ins.sync_info.on_wait.clear()
            probe.ins.sync_info.on_wait.append(waits[0])
            for w in waits[1:]:
                nop_inst = nc.sync.nop(nofuse=True, hint="drain_wait_split")
                if nop_inst.ins.sync_info is None:
                    nop_inst.ins.sync_info = mybir.SyncInfo(on_wait=[], on_update=[])
                nop_inst.ins.sync_info.on_wait.append(w)
        nc.sync.drain()  # SP already observed every sem above
        nc.all_engine_barrier()
        popped = nc._tile_sem_poison_stack.pop()
        assert popped is self._sem_poison
        # sem clears skipped: NRT reloads sem state per execution; verified
        # by repeated-call correctness checks in test.py

    TileContext._orig_drain_and_barrier = TileContext._drain_and_barrier
    TileContext._drain_and_barrier = _drain_and_barrier
    TileContext._drain_patched = True


# this walrus build has small per-instruction sync-wait budgets; split any
# excess waits onto same-engine nops placed just before the instruction
# (waiting earlier on the same engine stream is always safe).
_WAIT_LIMITS = {"DMACOPY": 1, "NOOP": 1, "DRAIN": 1, "TRIGGEREDCOPY": 1}
_DEFAULT_WAIT_LIMIT = 1


def _split_excess_waits(nc):
    import concourse.mybir as mybir

    ctr = 0
    for fn in nc.m.functions:
        for blk in fn.blocks:
            lst = blk.instructions
            out = []
            changed = False
            for inst in lst:
                si = inst.sync_info
                waits = list(si.on_wait) if si else []
                opname = type(inst).__name__.replace("Inst", "").upper()
                limit = _WAIT_LIMITS.get(opname, _DEFAULT_WAIT_LIMIT)
                if len(waits) > limit:
                    keep = waits[-limit:]
                    excess = waits[:-limit]
                    si.on_wait.clear()
                    for w in keep:
                        si.on_wait.append(w)
                    for w in excess:
                        nop = mybir.InstNoOp(name=f"WSPLIT-{ctr}", ins=[], outs=[])
                        ctr += 1
                        nop.engine = inst.engine
                        nop.sync_info = mybir.SyncInfo(on_wait=[w], on_update=[])
                        out.append(nop)
                    changed = True
                out.append(inst)
            if changed:
                lst[:] = out


def _build(mm_dtype_name=MM_DTYPE, for_sim=False):
    import concourse.bass as bass
    import concourse.mybir as mybir
    from concourse.tile import TileContext

    _patch_tile_drain()

    f32 = mybir.dt.float32
    mmdt = getattr(mybir.dt, mm_dtype_name)
    AF = mybir.ActivationFunctionType
    OP = mybir.AluOpType

    nc = bass.Bass(trn_type="TRN2")
    nc._bass_sim_build = for_sim

    # host-prepped inputs
    x_s = nc.dram_tensor("x_s", [BC, D + 2], f32, kind="ExternalInput")
    w1hat_h = nc.dram_tensor("w1hat_h", [D + 2, H], f32, kind="ExternalInput")
    w2_h = nc.dram_tensor("w2_h", [H, H], f32, kind="ExternalInput")
    g_h = nc.dram_tensor("g_h", [H, H], f32, kind="ExternalInput")
    w3_h = nc.dram_tensor("w3_h", [H, D], f32, kind="ExternalInput")
    # bias pack: col 0..KT-1 = b2 tiles, col KT = b3 (rows 0..63)
    bias_h = nc.dram_tensor("bias_h", [128, KT + 1], f32, kind="ExternalInput")
    # identity (mm dtype) cols 0..127; col 128 = ones
    idp_h = nc.dram_tensor("idp_h", [128, 129], mmdt, kind="ExternalInput")
    out = nc.dram_tensor("out", [BC, D + 1], f32, kind="ExternalOutput")

    def mm(ap):
        return ap.bitcast(mmdt) if mmdt != f32 else ap

    with TileContext(nc) as tc:
        with (
            tc.tile_pool(name="weights", bufs=1) as wpool,
            tc.tile_pool(name="acts", bufs=1) as apool,
            tc.tile_pool(name="xin", bufs=1) as xpool,
            tc.tile_pool(name="psmm", bufs=6, space="PSUM") as psmm,
            tc.tile_pool(name="pstr", bufs=2, space="PSUM") as pstr,
            tc.tile_pool(name="outbuf", bufs=1) as opool,
        ):
            # ---------------- input / weight DMAs --------------------------
            idp = wpool.tile([128, 129], mmdt)
            nc.sync.dma_start(out=idp, in_=idp_h[:])
            identr = idp[:, 0:128]
            ones_col = idp[:, 128:129]

            x_sb = xpool.tile([128, BT, D + 2], mmdt)
            half = BT // 2
            nc.sync.dma_start(
                out=x_sb[:, 0:half, :],
                in_=x_s[0 : half * 128]
                .rearrange("(bt p) c -> p bt c", p=128)
                .bitcast(mmdt),
            )
            nc.scalar.dma_start(
                out=x_sb[:, half:BT, :],
                in_=x_s[half * 128 : BC]
                .rearrange("(bt p) c -> p bt c", p=128)
                .bitcast(mmdt),
            )

            w1hat = wpool.tile([D + 2, H], mmdt)
            nc.sync.dma_start(out=w1hat, in_=w1hat_h[:].bitcast(mmdt))

            bias_t = wpool.tile([128, KT + 1], f32)
            nc.scalar.dma_start(out=bias_t, in_=bias_h[:])
            b2t = bias_t[:, 0:KT]
            b3t = bias_t[0:D, KT : KT + 1]

            w24 = wpool.tile([128, KT, H], mmdt)
            nc.sync.dma_start(
                out=w24, in_=w2_h[:].rearrange("(t p) m -> p t m", p=128).bitcast(mmdt)
            )
            g24 = wpool.tile([128, KT, H], mmdt)
            nc.scalar.dma_start(
                out=g24, in_=g_h[:].rearrange("(t p) m -> p t m", p=128).bitcast(mmdt)
            )
            w34 = wpool.tile([128, KT, D], mmdt)
            nc.sync.dma_start(
                out=w34, in_=w3_h[:].rearrange("(t p) m -> p t m", p=128).bitcast(mmdt)
            )

            # ---------------- xhatT assembly (per-chunk tiles) -------------
            # x_s columns 64/65 hold t and 1 (host-prepped), so each PE
            # transpose emits complete xhat rows [x^T; t*ones; ones]
            xhat = []
            for n in range(NCH):
                xh = apool.tile([D + 2, CH], mmdt, tag=f"xhat{n}", name=f"xhat{n}")
                xhat.append(xh)
                pti = pstr.tile([D + 2, 512], mmdt, tag="tr", name=f"pti{n}")
                for j in range(4):
                    bt = n * 4 + j
                    nc.tensor.transpose(
                        pti[:, j * 128 : (j + 1) * 128],
                        x_sb[:, bt, :],
                        identr,
                    )
                nc.vector.tensor_copy(xh, pti)

            # ---------------- L1: h1 = tanh(W1hat^T @ xhat) ----------------
            # m1 = h1^2 - 1 via DVE square (TT) + sub (TS), all SBUF
            h1 = [apool.tile([128, KT, CH], mmdt, tag=f"h1_{n}", name=f"h1_{n}") for n in range(NCH)]
            m1 = [apool.tile([128, KT, CH], mmdt, tag=f"m1_{n}", name=f"m1_{n}") for n in range(NCH)]
            sq1 = [apool.tile([128, KT, CH], mmdt, tag=f"sq1_{n}", name=f"sq1_{n}") for n in range(NCH)]
            for n in range(NCH):
                for i in range(KT):
                    pz = psmm.tile([128, CH], f32, tag="mmtile")
                    nc.tensor.matmul(
                        pz,
                        mm(w1hat[:, i * 128 : (i + 1) * 128]),
                        mm(xhat[n]),
                        start=True,
                        stop=True,
                    )
                    nc.scalar.activation(h1[n][:, i, :], pz, AF.Tanh)
                    nc.vector.tensor_mul(
                        sq1[n][:, i, :], h1[n][:, i, :], h1[n][:, i, :]
                    )
                    nc.vector.tensor_scalar_sub(
                        m1[n][:, i, :], sq1[n][:, i, :], 1.0
                    )

            # ---------------- L2: h2 = tanh(W2^T h1 + b2) ------------------
            h2 = [apool.tile([128, KT, CH], mmdt, tag=f"h2_{n}", name=f"h2_{n}") for n in range(NCH)]
            m2 = [apool.tile([128, KT, CH], mmdt, tag=f"m2_{n}", name=f"m2_{n}") for n in range(NCH)]
            sq2 = [apool.tile([128, KT, CH], mmdt, tag=f"sq2_{n}", name=f"sq2_{n}") for n in range(NCH)]
            for n in range(NCH):
                for i in range(KT):
                    pz = psmm.tile([128, CH], f32, tag="mmtile")
                    for k in range(KT):
                        nc.tensor.matmul(
                            pz,
                            mm(w24[:, k, i * 128 : (i + 1) * 128]),
                            mm(h1[n][:, k, :]),
                            start=(k == 0),
                            stop=(k == KT - 1),
                        )
                    nc.scalar.activation(
                        h2[n][:, i, :], pz, AF.Tanh, bias=b2t[:, i : i + 1]
                    )
                    nc.vector.tensor_mul(
                        sq2[n][:, i, :], h2[n][:, i, :], h2[n][:, i, :]
                    )
                    nc.vector.tensor_scalar_sub(
                        m2[n][:, i, :], sq2[n][:, i, :], 1.0
                    )

            # ---------------- c = G^T m1 ; e = m2 * c ; KT-tree ------------
            ebuf = [apool.tile([128, KT, CH], mmdt, tag=f"eb{n}", name=f"eb{n}") for n in range(NCH)]
            ered = [apool.tile([128, CH], mmdt, tag=f"er{n}", name=f"er{n}") for n in range(NCH)]
            for n in range(NCH):
                for i in range(KT):
                    pc = psmm.tile([128, CH], f32, tag="mmtile")
                    for k in range(KT):
                        nc.tensor.matmul(
                            pc,
                            mm(g24[:, k, i * 128 : (i + 1) * 128]),
                            mm(m1[n][:, k, :]),
                            start=(k == 0),
                            stop=(k == KT - 1),
                        )
                    nc.vector.tensor_mul(ebuf[n][:, i, :], m2[n][:, i, :], pc)
                # KT reduction tree: 2 pair-adds into ebuf slots, final into ered
                nc.vector.tensor_add(
                    ebuf[n][:, 0, :], ebuf[n][:, 0, :], ebuf[n][:, 1, :]
                )
                nc.vector.tensor_add(
                    ebuf[n][:, 2, :], ebuf[n][:, 2, :], ebuf[n][:, 3, :]
                )
                nc.vector.tensor_add(
                    ered[n], ebuf[n][:, 0, :], ebuf[n][:, 2, :]
                )

            # ---------------- L3 + div + output ----------------------------
            # odt: f32-bits for float32r (bitcast views in the transpose),
            # native mmdt otherwise (copy-convert to f32 at ob)
            f32r_like = mm_dtype_name == "float32r"
            odt = f32 if f32r_like else mmdt
            ob = opool.tile([128, BT, D + 1], f32)
            for n in range(NCH):
                outT = apool.tile([D + 1, CH], odt, tag=f"outT{n}", name=f"outT{n}")
                pd = psmm.tile([1, CH], f32, tag="mmtile", name=f"pd{n}")
                nc.tensor.matmul(
                    pd, mm(ones_col), mm(ered[n]), start=True, stop=True
                )
                nc.scalar.activation(outT[D : D + 1, :], pd, AF.Identity)
                po = psmm.tile([D, CH], f32, tag="mmtile", name=f"po{n}")
                for k in range(KT):
                    nc.tensor.matmul(
                        po,
                        mm(w34[:, k, :]),
                        mm(h2[n][:, k, :]),
                        start=(k == 0),
                        stop=(k == KT - 1),
                    )
                nc.scalar.activation(outT[0:D, :], po, AF.Identity, bias=b3t)

                pto = pstr.tile([128, 4, D + 1], odt, tag="tr", name=f"pto{n}")
                tr_id = identr.bitcast(f32) if f32r_like else identr
                for j in range(4):
                    nc.tensor.transpose(
                        pto[:, j, :],
                        outT[:, j * 128 : (j + 1) * 128],
                        tr_id[0 : D + 1, 0 : D + 1],
                    )
                nc.vector.tensor_copy(ob[:, n * 4 : (n + 1) * 4, :], pto)
                nc.sync.dma_start(
                    out=out[n * CH : (n + 1) * CH, :].rearrange(
                        "(bt p) c -> p bt c", p=128
                    ),
                    in_=ob[:, n * 4 : (n + 1) * 4, :],
                )

    if not for_sim:
        _split_excess_waits(nc)
    return nc


def _get_nc():
    if "nc" not in _CACHE:
        _CACHE["nc"] = _build()
    return _CACHE["nc"]


def _np_mmdt():
    import concourse.mybir as mybir

    return mybir.dt.np(getattr(mybir.dt, MM_DTYPE))


def _make_in_maps(inputs):
    t = np.asarray(inputs["t"], np.float32)
    x = np.asarray(inputs["x"], np.float32)
    W1 = np.asarray(inputs["W1"], np.float32)
    b1 = np.asarray(inputs["b1"], np.float32)
    W2 = np.asarray(inputs["W2"], np.float32)
    b2 = np.asarray(inputs["b2"], np.float32)
    W3 = np.asarray(inputs["W3"], np.float32)
    b3 = np.asarray(inputs["b3"], np.float32)

    # columns 64/65 carry t and 1 so the device transposes produce the
    # [x^T; t*ones; ones] contraction operand directly
    x_aug = np.empty((B, D + 2), np.float32)
    x_aug[:, 0:D] = x[:, 0:D]
    x_aug[:, D] = t[0]
    x_aug[:, D + 1] = 1.0

    w1hat = np.concatenate([W1[1:], W1[0:1], b1[None]], axis=0)  # [66, H]

    # host G = W2 * (W1[1:].T @ W3.T)   [H, H]
    G = (W2 * (W1[1:].T @ W3.T)).astype(np.float32)

    bias = np.zeros((128, KT + 1), np.float32)
    bias[:, 0:KT] = b2.reshape(KT, 128).T
    bias[0:D, KT] = b3

    if MM_DTYPE == "float32r":
        idp = np.zeros((128, 129), np.float32)
        idp[:, 0:128] = np.eye(128, dtype=np.float32)
        idp[:, 128] = 1.0
    else:
        npdt = _np_mmdt()
        idp = np.zeros((128, 129), npdt)
        idp[:, 0:128] = np.eye(128, dtype=np.float32).astype(npdt)
        idp[:, 128] = np.float32(1.0)

    base = {
        "w1hat_h": np.ascontiguousarray(w1hat),
        "w2_h": np.ascontiguousarray(W2),
        "g_h": np.ascontiguousarray(G),
        "w3_h": np.ascontiguousarray(W3),
        "bias_h": bias,
        "idp_h": idp,
    }
    return [
        dict(base, x_s=np.ascontiguousarray(x_aug[i * BC : (i + 1) * BC]))
        for i in range(N_CORES)
    ]


def _gather(res):
    return np.concatenate([res.results[i]["out"] for i in range(N_CORES)], axis=0)


def kernel(t, x, W1, b1, W2, b2, W3, b3):
    from concourse.bass_utils import run_bass_kernel_spmd

    nc = _get_nc()
    in_maps = _make_in_maps(
        dict(t=t, x=x, W1=W1, b1=b1, W2=W2, b2=b2, W3=W3, b3=b3)
    )
    res = run_bass_kernel_spmd(nc, in_maps, core_ids=list(range(N_CORES)))
    _CACHE["last_result"] = res
    out = _gather(res)
    # flaky-core guard: a dropped execution leaves the donated zero output
    # buffer untouched; the true output of this MLP is never all-zero.
    for _ in range(3):
        bad = [
            i
            for i in range(N_CORES)
            if not np.any(res.results[i]["out"][:, 0:D])
        ]
        if not bad:
            break
        res = run_bass_kernel_spmd(nc, in_maps, core_ids=list(range(N_CORES)))
        _CACHE["last_result"] = res
        out = _gather(res)
    return out


# revision 7
# speedup vs baseline: 2.1218x; 1.3089x over previous
"""CNF vector field + exact divergence kernel for Trainium2 (8 NeuronCores).

Math (per sample x of dim D=64, t scalar, 3-layer MLP 65->512->512->64):
    h1 = tanh(W1hat^T [x;t;1])       (w1hat rows: W1[1:], W1[0], b1)
    h2 = tanh(W2^T h1 + b2)
    dx = W3^T h2 + b3
    div = trace(d dx / d x) = (h1^2-1)^T G (h2^2-1)
    with G = W2 * (W1[1:].T @ W3.T)  computed on HOST (weights-only).

Layout: everything on device is feature-major ([feature partitions, batch
free]).  The host pre-transposes x into xhat^T = [x;t;1]^T (feature-major)
and transposes the [65, B] device output back to [B, 65] -- zero PE
transposes on device.

Per-core per-chunk (CH=512 batch columns, 2 chunks):
  L1  4 MMs  -> pz1 big PSUM [128,4,512] -> 1 big tanh -> h1
  DVE: sq1 = h1*h1 (big), m1 = sq1 - 1 (big)
  L2  16 MMs -> pz2 (per i-tile) -> tanh+bias b2 -> h2 (4 ACT)
  DVE: sq2 = h2*h2 (big), m2 = sq2 - 1 (big)
  c   16 MMs -> pc (per i-tile) -> e_i = m2_i * pc_i (DVE TT, PSUM read)
  div 4 ones-MMs accumulate pd[1,CH] over e k-tiles (no DVE tree)
  L3  4 MMs -> po -> ACT +b3 -> outT[0:64]; ACT pd -> outT[64]
  out DMA outT [65, CH] -> DRAM feature-major; host transposes back.

gpsimd does NOTHING (v1 measured 7.5us per elementwise op there).
"""

import sys

if "/opt/trn_rl_repo" not in sys.path:
    sys.path.insert(0, "/opt/trn_rl_repo")

import numpy as np

D = 64
H = 512
B = 8192
N_CORES = 8
BC = B // N_CORES          # 1024 samples per core
NCH = 2                    # batch chunks per core
CH = BC // NCH             # 512
KT = H // 128              # 4 k-tiles of the hidden dim

MM_DTYPE = "bfloat16"      # or "float32r"

_CACHE = {}


def _patch_tile_drain():
    """walrus in this toolchain accepts only one sync wait per CTRL
    instruction; split the TileContext tail-drain waits across nops."""
    import concourse.mybir as mybir
    from concourse.tile import TileContext
    from concourse.vector_clock import ScopedClock

    if getattr(TileContext, "_drain_patched", False):
        return

    def _drain_and_barrier(self, tick_clock, wait_clock):
        nc = self.nc
        probe = nc.sync.nop(nofuse=True, hint="drain_wait_probe")
        wait_clock.add_sem_waits(
            probe.ins, ScopedClock({None: tick_clock.global_clock})
        )
        waits = list(probe.ins.sync_info.on_wait) if probe.ins.sync_info else []
        if len(waits) > 1:
            probe.ins.sync_info.on_wait.clear()
            probe.ins.sync_info.on_wait.append(waits[0])
            for w in waits[1:]:
                nop_inst = nc.sync.nop(nofuse=True, hint="drain_wait_split")
                if nop_inst.ins.sync_info is None:
                    nop_inst.ins.sync_info = mybir.SyncInfo(on_wait=[], on_update=[])
                nop_inst.ins.sync_info.on_wait.append(w)
        nc.sync.drain()  # SP already observed every sem above
        nc.all_engine_barrier()
        popped = nc._tile_sem_poison_stack.pop()
        assert popped is self._sem_poison
        # sem clears skipped: NRT reloads sem state per execution; verified
        # by repeated-call correctness checks in test.py

    TileContext._orig_drain_and_barrier = TileContext._drain_and_barrier
    TileContext._drain_and_barrier = _drain_and_barrier
    TileContext._drain_patched = True


# this walrus build has small per-instruction sync-wait budgets; split any
# excess waits onto same-engine nops placed just before the instruction
# (waiting earlier on the same engine stream is always safe).
_WAIT_LIMITS = {"DMACOPY": 1, "NOOP": 1, "DRAIN": 1, "TRIGGEREDCOPY": 1}
_DEFAULT_WAIT_LIMIT = 1


def _split_excess_waits(nc):
    import concourse.mybir as mybir

    ctr = 0
    for fn in nc.m.functions:
        for blk in fn.blocks:
            lst = blk.instructions
            out = []
            changed = False
            for inst in lst:
                si = inst.sync_info
                waits = list(si.on_wait) if si else []
                opname = type(inst).__name__.replace("Inst", "").upper()
                limit = _WAIT_LIMITS.get(opname, _DEFAULT_WAIT_LIMIT)
                if len(waits) > limit:
                    keep = waits[-limit:]
                    excess = waits[:-limit]
                    si.on_wait.clear()
                    for w in keep:
                        si.on_wait.append(w)
                    for w in excess:
                        nop = mybir.InstNoOp(name=f"WSPLIT-{ctr}", ins=[], outs=[])
                        ctr += 1
                        nop.engine = inst.engine
                        nop.sync_info = mybir.SyncInfo(on_wait=[w], on_update=[])
                        out.append(nop)
                    changed = True
                out.append(inst)
            if changed:
                lst[:] = out


def _build(mm_dtype_name=MM_DTYPE, for_sim=False):
    import concourse.bass as bass
    import concourse.mybir as mybir
    from concourse.tile import TileContext

    _patch_tile_drain()

    f32 = mybir.dt.float32
    mmdt = getattr(mybir.dt, mm_dtype_name)
    AF = mybir.ActivationFunctionType

    f32r_like = mm_dtype_name in ("float32", "float32r")
    # dram dtype for matmul-operand tensors: f32 bits for f32r (bitcast
    # views), native mmdt (bf16) otherwise
    ddt = f32 if f32r_like else mmdt

    nc = bass.Bass(trn_type="TRN2")
    nc._bass_sim_build = for_sim

    # host-prepped inputs (feature-major)
    xhat_h = nc.dram_tensor("xhat_h", [D + 2, BC], ddt, kind="ExternalInput")
    w1hat_h = nc.dram_tensor("w1hat_h", [D + 2, H], ddt, kind="ExternalInput")
    w2_h = nc.dram_tensor("w2_h", [H, H], ddt, kind="ExternalInput")
    g_h = nc.dram_tensor("g_h", [H, H], ddt, kind="ExternalInput")
    w3_h = nc.dram_tensor("w3_h", [H, D], ddt, kind="ExternalInput")
    # bias pack (f32): col 0..KT-1 = b2 tiles, col KT = b3 (rows 0..63)
    bias_h = nc.dram_tensor("bias_h", [128, KT + 1], f32, kind="ExternalInput")
    ones_h = nc.dram_tensor("ones_h", [128, 1], ddt, kind="ExternalInput")
    out_f = nc.dram_tensor("out_f", [D + 1, BC], f32, kind="ExternalOutput")

    def dm(ap):
        # dram-side view for DMA into mmdt tiles (f32r is a bitcast of f32)
        return ap.bitcast(mmdt) if f32r_like else ap

    with TileContext(nc) as tc:
        with (
            tc.tile_pool(name="weights", bufs=1) as wpool,
            tc.tile_pool(name="acts", bufs=1) as apool,
            tc.tile_pool(name="psmm", bufs=7, space="PSUM") as psmm,
        ):
            # ---------------- input / weight DMAs --------------------------
            xh = apool.tile([D + 2, BC], mmdt)
            nc.sync.dma_start(out=xh, in_=dm(xhat_h[:]))
            w1hat = wpool.tile([D + 2, H], mmdt)
            nc.sync.dma_start(out=w1hat, in_=dm(w1hat_h[:]))
            ones_col = wpool.tile([128, 1], mmdt)
            nc.sync.dma_start(out=ones_col, in_=dm(ones_h[:]))
            bias_t = wpool.tile([128, KT + 1], f32)
            nc.scalar.dma_start(out=bias_t, in_=bias_h[:])
            b2t = bias_t[:, 0:KT]
            b3t = bias_t[0:D, KT : KT + 1]

            w24 = wpool.tile([128, KT, H], mmdt)
            nc.sync.dma_start(
                out=w24, in_=dm(w2_h[:].rearrange("(t p) m -> p t m", p=128))
            )
            g24 = wpool.tile([128, KT, H], mmdt)
            nc.scalar.dma_start(
                out=g24, in_=dm(g_h[:].rearrange("(t p) m -> p t m", p=128))
            )
            w34 = wpool.tile([128, KT, D], mmdt)
            nc.scalar.dma_start(
                out=w34, in_=dm(w3_h[:].rearrange("(t p) m -> p t m", p=128))
            )

            # per-chunk activation tiles
            h1 = [apool.tile([128, KT, CH], mmdt, tag=f"h1_{n}", name=f"h1_{n}") for n in range(NCH)]
            m1 = [apool.tile([128, KT, CH], mmdt, tag=f"m1_{n}", name=f"m1_{n}") for n in range(NCH)]
            sq1 = [apool.tile([128, KT, CH], mmdt, tag=f"sq1_{n}", name=f"sq1_{n}") for n in range(NCH)]
            h2 = [apool.tile([128, KT, CH], mmdt, tag=f"h2_{n}", name=f"h2_{n}") for n in range(NCH)]
            m2 = [apool.tile([128, KT, CH], mmdt, tag=f"m2_{n}", name=f"m2_{n}") for n in range(NCH)]
            sq2 = [apool.tile([128, KT, CH], mmdt, tag=f"sq2_{n}", name=f"sq2_{n}") for n in range(NCH)]
            ebuf = [apool.tile([128, KT, CH], mmdt, tag=f"eb{n}", name=f"eb{n}") for n in range(NCH)]

            for n in range(NCH):
                xslice = xh[:, n * CH : (n + 1) * CH]

                # -------- L1 -----------------------------------------------
                for i in range(KT):
                    pz = psmm.tile([128, CH], f32, tag="mmtile")
                    nc.tensor.matmul(
                        pz,
                        w1hat[:, i * 128 : (i + 1) * 128],
                        xslice,
                        start=True,
                        stop=True,
                    )
                    nc.scalar.activation(h1[n][:, i, :], pz, AF.Tanh)
                nc.vector.tensor_mul(sq1[n][:], h1[n][:], h1[n][:])
                nc.vector.tensor_scalar_sub(m1[n][:], sq1[n][:], 1.0)

                # -------- L2: per-i-tile (bias b2 varies per tile) ---------
                for i in range(KT):
                    pz = psmm.tile([128, CH], f32, tag="mmtile")
                    for k in range(KT):
                        nc.tensor.matmul(
                            pz,
                            w24[:, k, i * 128 : (i + 1) * 128],
                            h1[n][:, k, :],
                            start=(k == 0),
                            stop=(k == KT - 1),
                        )
                    nc.scalar.activation(
                        h2[n][:, i, :], pz, AF.Tanh, bias=b2t[:, i : i + 1]
                    )
                nc.vector.tensor_mul(sq2[n][:], h2[n][:], h2[n][:])
                nc.vector.tensor_scalar_sub(m2[n][:], sq2[n][:], 1.0)

                # -------- c = G^T m1 ; e_i = m2_i * pc_i -------------------
                for i in range(KT):
                    pc = psmm.tile([128, CH], f32, tag="mmtile")
                    for k in range(KT):
                        nc.tensor.matmul(
                            pc,
                            g24[:, k, i * 128 : (i + 1) * 128],
                            m1[n][:, k, :],
                            start=(k == 0),
                            stop=(k == KT - 1),
                        )
                    nc.vector.tensor_mul(ebuf[n][:, i, :], m2[n][:, i, :], pc)

                # -------- div: 4 accumulating ones-MMs over e k-tiles ------
                outT = apool.tile([D + 1, CH], f32, tag=f"outT{n}", name=f"outT{n}")
                pd = psmm.tile([1, CH], f32, tag="mmtile", name=f"pd{n}")
                for k in range(KT):
                    nc.tensor.matmul(
                        pd,
                        ones_col,
                        ebuf[n][:, k, :],
                        start=(k == 0),
                        stop=(k == KT - 1),
                    )
                nc.scalar.activation(outT[D : D + 1, :], pd, AF.Identity)

                # -------- L3 ----------------------------------------------
                po = psmm.tile([D, CH], f32, tag="mmtile", name=f"po{n}")
                for k in range(KT):
                    nc.tensor.matmul(
                        po,
                        w34[:, k, :],
                        h2[n][:, k, :],
                        start=(k == 0),
                        stop=(k == KT - 1),
                    )
                nc.scalar.activation(outT[0:D, :], po, AF.Identity, bias=b3t)

                nc.sync.dma_start(
                    out=out_f[:, n * CH : (n + 1) * CH], in_=outT
                )

    if not for_sim:
        _split_excess_waits(nc)
    return nc


def _get_nc():
    if "nc" not in _CACHE:
        _CACHE["nc"] = _build()
    return _CACHE["nc"]


def _np_ddt():
    import concourse.mybir as mybir

    if MM_DTYPE in ("float32", "float32r"):
        return np.float32
    return mybir.dt.np(getattr(mybir.dt, MM_DTYPE))


def _make_in_maps(inputs):
    t = np.asarray(inputs["t"], np.float32)
    x = np.asarray(inputs["x"], np.float32)
    W1 = np.asarray(inputs["W1"], np.float32)
    b1 = np.asarray(inputs["b1"], np.float32)
    W2 = np.asarray(inputs["W2"], np.float32)
    b2 = np.asarray(inputs["b2"], np.float32)
    W3 = np.asarray(inputs["W3"], np.float32)
    b3 = np.asarray(inputs["b3"], np.float32)
    ddt = _np_ddt()

    # feature-major xhat: rows 0..63 = x^T, row 64 = t, row 65 = 1
    xhat = np.empty((D + 2, B), np.float32)
    xhat[0:D] = x[:, 0:D].T
    xhat[D] = t[0]
    xhat[D + 1] = 1.0
    xhat = xhat.astype(ddt)

    w1hat = np.concatenate([W1[1:], W1[0:1], b1[None]], axis=0)  # [66, H]

    # host G = W2 * (W1[1:].T @ W3.T)   [H, H]
    G = (W2 * (W1[1:].T @ W3.T)).astype(np.float32)

    bias = np.zeros((128, KT + 1), np.float32)
    bias[:, 0:KT] = b2.reshape(KT, 128).T
    bias[0:D, KT] = b3

    base = {
        "w1hat_h": np.ascontiguousarray(w1hat.astype(ddt)),
        "w2_h": np.ascontiguousarray(W2.astype(ddt)),
        "g_h": np.ascontiguousarray(G.astype(ddt)),
        "w3_h": np.ascontiguousarray(W3.astype(ddt)),
        "bias_h": bias,
        "ones_h": np.ones((128, 1), ddt),
    }
    return [
        dict(base, xhat_h=np.ascontiguousarray(xhat[:, i * BC : (i + 1) * BC]))
        for i in range(N_CORES)
    ]


def _gather(res):
    # device output is feature-major [65, BC]; transpose back per core
    return np.concatenate(
        [np.ascontiguousarray(res.results[i]["out_f"].T) for i in range(N_CORES)],
        axis=0,
    )


def kernel(t, x, W1, b1, W2, b2, W3, b3):
    from concourse.bass_utils import run_bass_kernel_spmd

    nc = _get_nc()
    in_maps = _make_in_maps(
        dict(t=t, x=x, W1=W1, b1=b1, W2=W2, b2=b2, W3=W3, b3=b3)
    )
    res = run_bass_kernel_spmd(nc, in_maps, core_ids=list(range(N_CORES)))
    _CACHE["last_result"] = res
    out = _gather(res)
    # flaky-core guard: a dropped execution leaves the donated zero output
    # buffer untouched; the true output of this MLP is never all-zero.
    for _ in range(3):
        bad = [
            i
            for i in range(N_CORES)
            if not np.any(res.results[i]["out_f"][0:D, :])
        ]
        if not bad:
            break
        res = run_bass_kernel_spmd(nc, in_maps, core_ids=list(range(N_CORES)))
        _CACHE["last_result"] = res
        out = _gather(res)
    return out


# revision 10
# speedup vs baseline: 2.1604x; 1.0182x over previous
"""CNF vector field + exact divergence kernel for Trainium2 (8 NeuronCores).

Math (per sample x of dim D=64, t scalar, 3-layer MLP 65->512->512->64):
    h1 = tanh(W1hat^T [x;t;1])       (w1hat rows: W1[1:], W1[0], b1)
    h2 = tanh(W2^T h1 + b2)
    dx = W3^T h2 + b3
    div = trace(d dx / d x) = (h1^2-1)^T G (h2^2-1)
    with G = W2 * (W1[1:].T @ W3.T)  computed on HOST (weights-only).

Layout: everything on device is feature-major ([feature partitions, batch
free]).  The host pre-transposes x into xhat^T = [x;t;1]^T (feature-major)
and transposes the [65, B] device output back to [B, 65] -- zero PE
transposes on device.

Per-core per-chunk (CH=512 batch columns, 2 chunks):
  L1  4 MMs  -> pz1 big PSUM [128,4,512] -> 1 big tanh -> h1
  DVE: sq1 = h1*h1 (big), m1 = sq1 - 1 (big)
  L2  16 MMs -> pz2 (per i-tile) -> tanh+bias b2 -> h2 (4 ACT)
  DVE: sq2 = h2*h2 (big), m2 = sq2 - 1 (big)
  c   16 MMs -> pc (per i-tile) -> e_i = m2_i * pc_i (DVE TT, PSUM read)
  div 4 ones-MMs accumulate pd[1,CH] over e k-tiles (no DVE tree)
  L3  4 MMs -> po -> ACT +b3 -> outT[0:64]; ACT pd -> outT[64]
  out DMA outT [65, CH] -> DRAM feature-major; host transposes back.

gpsimd does NOTHING (v1 measured 7.5us per elementwise op there).
"""

import sys

if "/opt/trn_rl_repo" not in sys.path:
    sys.path.insert(0, "/opt/trn_rl_repo")

import numpy as np

D = 64
H = 512
B = 8192
N_CORES = 8
BC = B // N_CORES          # 1024 samples per core
NCH = 2                    # batch chunks per core
CH = BC // NCH             # 512
KT = H // 128              # 4 k-tiles of the hidden dim

MM_DTYPE = "bfloat16"      # or "float32r"

_CACHE = {}


def _patch_tile_drain():
    """walrus in this toolchain accepts only one sync wait per CTRL
    instruction; split the TileContext tail-drain waits across nops."""
    import concourse.mybir as mybir
    from concourse.tile import TileContext
    from concourse.vector_clock import ScopedClock

    if getattr(TileContext, "_drain_patched", False):
        return

    def _drain_and_barrier(self, tick_clock, wait_clock):
        nc = self.nc
        probe = nc.sync.nop(nofuse=True, hint="drain_wait_probe")
        wait_clock.add_sem_waits(
            probe.ins, ScopedClock({None: tick_clock.global_clock})
        )
        waits = list(probe.ins.sync_info.on_wait) if probe.ins.sync_info else []
        if len(waits) > 1:
            probe.ins.sync_info.on_wait.clear()
            probe.ins.sync_info.on_wait.append(waits[0])
            for w in waits[1:]:
                nop_inst = nc.sync.nop(nofuse=True, hint="drain_wait_split")
                if nop_inst.ins.sync_info is None:
                    nop_inst.ins.sync_info = mybir.SyncInfo(on_wait=[], on_update=[])
                nop_inst.ins.sync_info.on_wait.append(w)
        nc.sync.drain()  # SP already observed every sem above
        nc.all_engine_barrier()
        popped = nc._tile_sem_poison_stack.pop()
        assert popped is self._sem_poison
        # sem clears skipped: NRT reloads sem state per execution; verified
        # by repeated-call correctness checks in test.py

    TileContext._orig_drain_and_barrier = TileContext._drain_and_barrier
    TileContext._drain_and_barrier = _drain_and_barrier
    TileContext._drain_patched = True


# this walrus build has small per-instruction sync-wait budgets; split any
# excess waits onto same-engine nops placed just before the instruction
# (waiting earlier on the same engine stream is always safe).
_WAIT_LIMITS = {"DMACOPY": 1, "NOOP": 1, "DRAIN": 1, "TRIGGEREDCOPY": 1}
_DEFAULT_WAIT_LIMIT = 1


def _split_excess_waits(nc):
    import concourse.mybir as mybir

    ctr = 0
    for fn in nc.m.functions:
        for blk in fn.blocks:
            lst = blk.instructions
            out = []
            changed = False
            for inst in lst:
                si = inst.sync_info
                waits = list(si.on_wait) if si else []
                opname = type(inst).__name__.replace("Inst", "").upper()
                limit = _WAIT_LIMITS.get(opname, _DEFAULT_WAIT_LIMIT)
                if len(waits) > limit:
                    keep = waits[-limit:]
                    excess = waits[:-limit]
                    si.on_wait.clear()
                    for w in keep:
                        si.on_wait.append(w)
                    for w in excess:
                        nop = mybir.InstNoOp(name=f"WSPLIT-{ctr}", ins=[], outs=[])
                        ctr += 1
                        nop.engine = inst.engine
                        nop.sync_info = mybir.SyncInfo(on_wait=[w], on_update=[])
                        out.append(nop)
                    changed = True
                out.append(inst)
            if changed:
                lst[:] = out


def _build(mm_dtype_name=MM_DTYPE, for_sim=False):
    import concourse.bass as bass
    import concourse.mybir as mybir
    from concourse.tile import TileContext

    _patch_tile_drain()

    f32 = mybir.dt.float32
    mmdt = getattr(mybir.dt, mm_dtype_name)
    AF = mybir.ActivationFunctionType

    f32r_like = mm_dtype_name in ("float32", "float32r")
    # dram dtype for matmul-operand tensors: f32 bits for f32r (bitcast
    # views), native mmdt (bf16) otherwise
    ddt = f32 if f32r_like else mmdt

    nc = bass.Bass(trn_type="TRN2")
    nc._bass_sim_build = for_sim

    # host-prepped inputs (feature-major)
    xhat_h = nc.dram_tensor("xhat_h", [D + 2, BC], ddt, kind="ExternalInput")
    w1hat_h = nc.dram_tensor("w1hat_h", [D + 2, H], ddt, kind="ExternalInput")
    w2_h = nc.dram_tensor("w2_h", [H, H], ddt, kind="ExternalInput")
    g_h = nc.dram_tensor("g_h", [H, H], ddt, kind="ExternalInput")
    w3_h = nc.dram_tensor("w3_h", [H, D], ddt, kind="ExternalInput")
    # bias pack (f32): col 0..KT-1 = b2 tiles, col KT = b3 (rows 0..63)
    bias_h = nc.dram_tensor("bias_h", [128, KT + 1], f32, kind="ExternalInput")
    ones_h = nc.dram_tensor("ones_h", [128, 1], ddt, kind="ExternalInput")
    out_f = nc.dram_tensor("out_f", [D + 1, BC], f32, kind="ExternalOutput")

    def dm(ap):
        # dram-side view for DMA into mmdt tiles (f32r is a bitcast of f32)
        return ap.bitcast(mmdt) if f32r_like else ap

    with TileContext(nc) as tc:
        with (
            tc.tile_pool(name="weights", bufs=1) as wpool,
            tc.tile_pool(name="acts", bufs=1) as apool,
            tc.tile_pool(name="psmm", bufs=7, space="PSUM") as psmm,
        ):
            # ---------------- input / weight DMAs --------------------------
            # queues: sync = critical path (x chunk0, w1hat, rest of x);
            # vector = w24 (needed at L2); scalar = bias+g24+w34 (later)
            xh = apool.tile([D + 2, BC], mmdt)
            w1hat = wpool.tile([D + 2, H], mmdt)
            nc.sync.dma_start(
                out=xh[:, 0:CH], in_=dm(xhat_h[:, 0:CH])
            )
            nc.sync.dma_start(out=w1hat, in_=dm(w1hat_h[:]))
            w24 = wpool.tile([128, KT, H], mmdt)
            nc.sync.dma_start(
                out=w24, in_=dm(w2_h[:].rearrange("(t p) m -> p t m", p=128))
            )
            nc.sync.dma_start(
                out=xh[:, CH:BC], in_=dm(xhat_h[:, CH:BC])
            )
            ones_col = wpool.tile([128, 1], mmdt)
            nc.sync.dma_start(out=ones_col, in_=dm(ones_h[:]))
            bias_t = wpool.tile([128, KT + 1], f32)
            nc.scalar.dma_start(out=bias_t, in_=bias_h[:])
            b2t = bias_t[:, 0:KT]
            b3t = bias_t[0:D, KT : KT + 1]
            g24 = wpool.tile([128, KT, H], mmdt)
            nc.scalar.dma_start(
                out=g24, in_=dm(g_h[:].rearrange("(t p) m -> p t m", p=128))
            )
            w34 = wpool.tile([128, KT, D], mmdt)
            nc.scalar.dma_start(
                out=w34, in_=dm(w3_h[:].rearrange("(t p) m -> p t m", p=128))
            )

            # per-chunk activation tiles
            h1 = [apool.tile([128, KT, CH], mmdt, tag=f"h1_{n}", name=f"h1_{n}") for n in range(NCH)]
            m1 = [apool.tile([128, KT, CH], mmdt, tag=f"m1_{n}", name=f"m1_{n}") for n in range(NCH)]
            sq1 = [apool.tile([128, KT, CH], mmdt, tag=f"sq1_{n}", name=f"sq1_{n}") for n in range(NCH)]
            h2 = [apool.tile([128, KT, CH], mmdt, tag=f"h2_{n}", name=f"h2_{n}") for n in range(NCH)]
            m2 = [apool.tile([128, KT, CH], mmdt, tag=f"m2_{n}", name=f"m2_{n}") for n in range(NCH)]
            sq2 = [apool.tile([128, KT, CH], mmdt, tag=f"sq2_{n}", name=f"sq2_{n}") for n in range(NCH)]
            ebuf = [apool.tile([128, KT, CH], mmdt, tag=f"eb{n}", name=f"eb{n}") for n in range(NCH)]

            for n in range(NCH):
                xslice = xh[:, n * CH : (n + 1) * CH]

                # -------- L1 -----------------------------------------------
                for i in range(KT):
                    pz = psmm.tile([128, CH], f32, tag="mmtile")
                    nc.tensor.matmul(
                        pz,
                        w1hat[:, i * 128 : (i + 1) * 128],
                        xslice,
                        start=True,
                        stop=True,
                    )
                    nc.scalar.activation(h1[n][:, i, :], pz, AF.Tanh)
                nc.vector.tensor_mul(sq1[n][:], h1[n][:], h1[n][:])
                nc.vector.tensor_scalar_sub(m1[n][:], sq1[n][:], 1.0)

                # -------- L2: per-i-tile (bias b2 varies per tile) ---------
                # sq2/m2 per-i-tile so e_i unlocks as soon as tanh2_i lands
                for i in range(KT):
                    pz = psmm.tile([128, CH], f32, tag="mmtile")
                    for k in range(KT):
                        nc.tensor.matmul(
                            pz,
                            w24[:, k, i * 128 : (i + 1) * 128],
                            h1[n][:, k, :],
                            start=(k == 0),
                            stop=(k == KT - 1),
                        )
                    nc.scalar.activation(
                        h2[n][:, i, :], pz, AF.Tanh, bias=b2t[:, i : i + 1]
                    )
                    nc.vector.tensor_mul(
                        sq2[n][:, i, :], h2[n][:, i, :], h2[n][:, i, :]
                    )
                    nc.vector.tensor_scalar_sub(
                        m2[n][:, i, :], sq2[n][:, i, :], 1.0
                    )

                # -------- L3 (only needs h2; PE work while DVE does e) -----
                outT = apool.tile([D + 1, CH], f32, tag=f"outT{n}", name=f"outT{n}")
                po = psmm.tile([D, CH], f32, tag="mmtile", name=f"po{n}")
                for k in range(KT):
                    nc.tensor.matmul(
                        po,
                        w34[:, k, :],
                        h2[n][:, k, :],
                        start=(k == 0),
                        stop=(k == KT - 1),
                    )
                nc.scalar.activation(outT[0:D, :], po, AF.Identity, bias=b3t)

                # -------- c = G^T m1 ; e_i = m2_i * pc_i -------------------
                for i in range(KT):
                    pc = psmm.tile([128, CH], f32, tag="mmtile")
                    for k in range(KT):
                        nc.tensor.matmul(
                            pc,
                            g24[:, k, i * 128 : (i + 1) * 128],
                            m1[n][:, k, :],
                            start=(k == 0),
                            stop=(k == KT - 1),
                        )
                    nc.vector.tensor_mul(ebuf[n][:, i, :], m2[n][:, i, :], pc)

                # -------- div: 4 accumulating ones-MMs over e k-tiles ------
                pd = psmm.tile([1, CH], f32, tag="mmtile", name=f"pd{n}")
                for k in range(KT):
                    nc.tensor.matmul(
                        pd,
                        ones_col,
                        ebuf[n][:, k, :],
                        start=(k == 0),
                        stop=(k == KT - 1),
                    )
                nc.scalar.activation(outT[D : D + 1, :], pd, AF.Identity)

                nc.sync.dma_start(
                    out=out_f[:, n * CH : (n + 1) * CH], in_=outT
                )

    if not for_sim:
        _split_excess_waits(nc)
    return nc


def _get_nc():
    if "nc" not in _CACHE:
        _CACHE["nc"] = _build()
    return _CACHE["nc"]


def _np_ddt():
    import concourse.mybir as mybir

    if MM_DTYPE in ("float32", "float32r"):
        return np.float32
    return mybir.dt.np(getattr(mybir.dt, MM_DTYPE))


def _make_in_maps(inputs):
    t = np.asarray(inputs["t"], np.float32)
    x = np.asarray(inputs["x"], np.float32)
    W1 = np.asarray(inputs["W1"], np.float32)
    b1 = np.asarray(inputs["b1"], np.float32)
    W2 = np.asarray(inputs["W2"], np.float32)
    b2 = np.asarray(inputs["b2"], np.float32)
    W3 = np.asarray(inputs["W3"], np.float32)
    b3 = np.asarray(inputs["b3"], np.float32)
    ddt = _np_ddt()

    # feature-major xhat: rows 0..63 = x^T, row 64 = t, row 65 = 1
    xhat = np.empty((D + 2, B), np.float32)
    xhat[0:D] = x[:, 0:D].T
    xhat[D] = t[0]
    xhat[D + 1] = 1.0
    xhat = xhat.astype(ddt)

    w1hat = np.concatenate([W1[1:], W1[0:1], b1[None]], axis=0)  # [66, H]

    # host G = W2 * (W1[1:].T @ W3.T)   [H, H]
    G = (W2 * (W1[1:].T @ W3.T)).astype(np.float32)

    bias = np.zeros((128, KT + 1), np.float32)
    bias[:, 0:KT] = b2.reshape(KT, 128).T
    bias[0:D, KT] = b3

    base = {
        "w1hat_h": np.ascontiguousarray(w1hat.astype(ddt)),
        "w2_h": np.ascontiguousarray(W2.astype(ddt)),
        "g_h": np.ascontiguousarray(G.astype(ddt)),
        "w3_h": np.ascontiguousarray(W3.astype(ddt)),
        "bias_h": bias,
        "ones_h": np.ones((128, 1), ddt),
    }
    return [
        dict(base, xhat_h=np.ascontiguousarray(xhat[:, i * BC : (i + 1) * BC]))
        for i in range(N_CORES)
    ]


def _gather(res):
    # device output is feature-major [65, BC]; transpose back per core
    return np.concatenate(
        [np.ascontiguousarray(res.results[i]["out_f"].T) for i in range(N_CORES)],
        axis=0,
    )


def kernel(t, x, W1, b1, W2, b2, W3, b3):
    from concourse.bass_utils import run_bass_kernel_spmd

    nc = _get_nc()
    in_maps = _make_in_maps(
        dict(t=t, x=x, W1=W1, b1=b1, W2=W2, b2=b2, W3=W3, b3=b3)
    )
    res = run_bass_kernel_spmd(nc, in_maps, core_ids=list(range(N_CORES)))
    _CACHE["last_result"] = res
    out = _gather(res)
    # flaky-core guard: a dropped execution leaves the donated zero output
    # buffer untouched; the true output of this MLP is never all-zero.
    for _ in range(3):
        bad = [
            i
            for i in range(N_CORES)
            if not np.any(res.results[i]["out_f"][0:D, :])
        ]
        if not bad:
            break
        res = run_bass_kernel_spmd(nc, in_maps, core_ids=list(range(N_CORES)))
        _CACHE["last_result"] = res
        out = _gather(res)
    return out


# revision 12
# speedup vs baseline: 2.1886x; 1.0131x over previous
"""CNF vector field + exact divergence kernel for Trainium2 (8 NeuronCores).

Math (per sample x of dim D=64, t scalar, 3-layer MLP 65->512->512->64):
    h1 = tanh(W1hat^T [x;t;1])       (w1hat rows: W1[1:], W1[0], b1)
    h2 = tanh(W2^T h1 + b2)
    dx = W3^T h2 + b3
    div = trace(d dx / d x) = (h1^2-1)^T G (h2^2-1)
    with G = W2 * (W1[1:].T @ W3.T)  computed on HOST (weights-only).

Layout: everything on device is feature-major ([feature partitions, batch
free]).  The host pre-transposes x into xhat^T = [x;t;1]^T (feature-major)
and transposes the [65, B] device output back to [B, 65] -- zero PE
transposes on device.

Per-core per-chunk (CH=512 batch columns, 2 chunks):
  L1  4 MMs  -> pz1 big PSUM [128,4,512] -> 1 big tanh -> h1
  DVE: sq1 = h1*h1 (big), m1 = sq1 - 1 (big)
  L2  16 MMs -> pz2 (per i-tile) -> tanh+bias b2 -> h2 (4 ACT)
  DVE: sq2 = h2*h2 (big), m2 = sq2 - 1 (big)
  c   16 MMs -> pc (per i-tile) -> e_i = m2_i * pc_i (DVE TT, PSUM read)
  div 4 ones-MMs accumulate pd[1,CH] over e k-tiles (no DVE tree)
  L3  4 MMs -> po -> ACT +b3 -> outT[0:64]; ACT pd -> outT[64]
  out DMA outT [65, CH] -> DRAM feature-major; host transposes back.

gpsimd does NOTHING (v1 measured 7.5us per elementwise op there).
"""

import sys

if "/opt/trn_rl_repo" not in sys.path:
    sys.path.insert(0, "/opt/trn_rl_repo")

import numpy as np

D = 64
H = 512
B = 8192
N_CORES = 8
BC = B // N_CORES          # 1024 samples per core
NCH = 2                    # batch chunks per core
CH = BC // NCH             # 512
KT = H // 128              # 4 k-tiles of the hidden dim

MM_DTYPE = "bfloat16"      # or "float32r"

_CACHE = {}


def _patch_tile_drain():
    """walrus in this toolchain accepts only one sync wait per CTRL
    instruction; split the TileContext tail-drain waits across nops."""
    import concourse.mybir as mybir
    from concourse.tile import TileContext
    from concourse.vector_clock import ScopedClock

    if getattr(TileContext, "_drain_patched", False):
        return

    def _drain_and_barrier(self, tick_clock, wait_clock):
        # Distribute the tail sem-waits across all engines (walrus accepts
        # only one wait per instruction, so serial SP nops cost ~3us) and
        # skip the cross-engine EVSEM barrier: each engine stream simply
        # ends once its waits are satisfied; NRT completion requires all
        # engine queues + DMA queues done, which the drains cover.
        nc = self.nc
        probe = nc.sync.nop(nofuse=True, hint="drain_wait_probe")
        wait_clock.add_sem_waits(
            probe.ins, ScopedClock({None: tick_clock.global_clock})
        )
        waits = list(probe.ins.sync_info.on_wait) if probe.ins.sync_info else []
        if probe.ins.sync_info is not None:
            probe.ins.sync_info.on_wait.clear()
        engines = [nc.sync, nc.scalar, nc.vector, nc.tensor, nc.gpsimd]
        for idx, w in enumerate(waits):
            eng = engines[idx % len(engines)]
            nop_inst = eng.nop(nofuse=True, hint=f"drain_wait_{idx}")
            if nop_inst.ins.sync_info is None:
                nop_inst.ins.sync_info = mybir.SyncInfo(on_wait=[], on_update=[])
            nop_inst.ins.sync_info.on_wait.append(w)
        nc.sync.drain()
        nc.scalar.drain()  # both DMA-issuing engines drain their queues
        popped = nc._tile_sem_poison_stack.pop()
        assert popped is self._sem_poison
        # sem clears skipped: NRT reloads sem state per execution; verified
        # by repeated-call correctness checks in test.py

    TileContext._orig_drain_and_barrier = TileContext._drain_and_barrier
    TileContext._drain_and_barrier = _drain_and_barrier
    TileContext._drain_patched = True


# this walrus build has small per-instruction sync-wait budgets; split any
# excess waits onto same-engine nops placed just before the instruction
# (waiting earlier on the same engine stream is always safe).
_WAIT_LIMITS = {"DMACOPY": 1, "NOOP": 1, "DRAIN": 1, "TRIGGEREDCOPY": 1}
_DEFAULT_WAIT_LIMIT = 1


def _split_excess_waits(nc):
    import concourse.mybir as mybir

    ctr = 0
    for fn in nc.m.functions:
        for blk in fn.blocks:
            lst = blk.instructions
            out = []
            changed = False
            for inst in lst:
                si = inst.sync_info
                waits = list(si.on_wait) if si else []
                opname = type(inst).__name__.replace("Inst", "").upper()
                limit = _WAIT_LIMITS.get(opname, _DEFAULT_WAIT_LIMIT)
                if len(waits) > limit:
                    keep = waits[-limit:]
                    excess = waits[:-limit]
                    si.on_wait.clear()
                    for w in keep:
                        si.on_wait.append(w)
                    for w in excess:
                        nop = mybir.InstNoOp(name=f"WSPLIT-{ctr}", ins=[], outs=[])
                        ctr += 1
                        nop.engine = inst.engine
                        nop.sync_info = mybir.SyncInfo(on_wait=[w], on_update=[])
                        out.append(nop)
                    changed = True
                out.append(inst)
            if changed:
                lst[:] = out


def _build(mm_dtype_name=MM_DTYPE, for_sim=False):
    import concourse.bass as bass
    import concourse.mybir as mybir
    from concourse.tile import TileContext

    _patch_tile_drain()

    f32 = mybir.dt.float32
    mmdt = getattr(mybir.dt, mm_dtype_name)
    AF = mybir.ActivationFunctionType

    f32r_like = mm_dtype_name in ("float32", "float32r")
    # dram dtype for matmul-operand tensors: f32 bits for f32r (bitcast
    # views), native mmdt (bf16) otherwise
    ddt = f32 if f32r_like else mmdt

    nc = bass.Bass(trn_type="TRN2")
    nc._bass_sim_build = for_sim

    # host-prepped inputs (feature-major)
    xhat_h = nc.dram_tensor("xhat_h", [D + 2, BC], ddt, kind="ExternalInput")
    w1hat_h = nc.dram_tensor("w1hat_h", [D + 2, H], ddt, kind="ExternalInput")
    w2_h = nc.dram_tensor("w2_h", [H, H], ddt, kind="ExternalInput")
    g_h = nc.dram_tensor("g_h", [H, H], ddt, kind="ExternalInput")
    w3_h = nc.dram_tensor("w3_h", [H, D], ddt, kind="ExternalInput")
    # bias pack (f32): col 0..KT-1 = b2 tiles, col KT = b3 (rows 0..63)
    bias_h = nc.dram_tensor("bias_h", [128, KT + 1], f32, kind="ExternalInput")
    ones_h = nc.dram_tensor("ones_h", [128, 1], ddt, kind="ExternalInput")
    out_f = nc.dram_tensor("out_f", [D + 1, BC], f32, kind="ExternalOutput")

    def dm(ap):
        # dram-side view for DMA into mmdt tiles (f32r is a bitcast of f32)
        return ap.bitcast(mmdt) if f32r_like else ap

    with TileContext(nc) as tc:
        with (
            tc.tile_pool(name="weights", bufs=1) as wpool,
            tc.tile_pool(name="acts", bufs=1) as apool,
            tc.tile_pool(name="psmm", bufs=7, space="PSUM") as psmm,
        ):
            # ---------------- input / weight DMAs --------------------------
            # sync queue carries the critical path in need-order; scalar
            # queue starts with a dummy tanh so walrus emits the ACT table
            # load immediately (instead of right before the first real tanh)
            dummy = wpool.tile([1, 1], f32)
            nc.scalar.activation(
                dummy, nc.const_aps.scalar_like(1.0, dummy), AF.Tanh
            )

            xh = apool.tile([D + 2, BC], mmdt)
            w1hat = wpool.tile([D + 2, H], mmdt)
            nc.sync.dma_start(
                out=xh[:, 0:CH], in_=dm(xhat_h[:, 0:CH])
            )
            nc.sync.dma_start(out=w1hat, in_=dm(w1hat_h[:]))
            w24 = wpool.tile([128, KT, H], mmdt)
            nc.sync.dma_start(
                out=w24, in_=dm(w2_h[:].rearrange("(t p) m -> p t m", p=128))
            )
            nc.sync.dma_start(
                out=xh[:, CH:BC], in_=dm(xhat_h[:, CH:BC])
            )
            ones_col = wpool.tile([128, 1], mmdt)
            nc.sync.dma_start(out=ones_col, in_=dm(ones_h[:]))
            bias_t = wpool.tile([128, KT + 1], f32)
            nc.scalar.dma_start(out=bias_t, in_=bias_h[:])
            b2t = bias_t[:, 0:KT]
            b3t = bias_t[0:D, KT : KT + 1]
            g24 = wpool.tile([128, KT, H], mmdt)
            nc.scalar.dma_start(
                out=g24, in_=dm(g_h[:].rearrange("(t p) m -> p t m", p=128))
            )
            w34 = wpool.tile([128, KT, D], mmdt)
            nc.scalar.dma_start(
                out=w34, in_=dm(w3_h[:].rearrange("(t p) m -> p t m", p=128))
            )

            # per-chunk activation tiles
            h1 = [apool.tile([128, KT, CH], mmdt, tag=f"h1_{n}", name=f"h1_{n}") for n in range(NCH)]
            m1 = [apool.tile([128, KT, CH], mmdt, tag=f"m1_{n}", name=f"m1_{n}") for n in range(NCH)]
            sq1 = [apool.tile([128, KT, CH], mmdt, tag=f"sq1_{n}", name=f"sq1_{n}") for n in range(NCH)]
            h2 = [apool.tile([128, KT, CH], mmdt, tag=f"h2_{n}", name=f"h2_{n}") for n in range(NCH)]
            m2 = [apool.tile([128, KT, CH], mmdt, tag=f"m2_{n}", name=f"m2_{n}") for n in range(NCH)]
            sq2 = [apool.tile([128, KT, CH], mmdt, tag=f"sq2_{n}", name=f"sq2_{n}") for n in range(NCH)]
            ebuf = [apool.tile([128, KT, CH], mmdt, tag=f"eb{n}", name=f"eb{n}") for n in range(NCH)]

            for n in range(NCH):
                xslice = xh[:, n * CH : (n + 1) * CH]

                # -------- L1 -----------------------------------------------
                for i in range(KT):
                    pz = psmm.tile([128, CH], f32, tag="mmtile")
                    nc.tensor.matmul(
                        pz,
                        w1hat[:, i * 128 : (i + 1) * 128],
                        xslice,
                        start=True,
                        stop=True,
                    )
                    nc.scalar.activation(h1[n][:, i, :], pz, AF.Tanh)
                nc.vector.tensor_mul(sq1[n][:], h1[n][:], h1[n][:])
                nc.vector.tensor_scalar_sub(m1[n][:], sq1[n][:], 1.0)

                # -------- L2: per-i-tile (bias b2 varies per tile) ---------
                # sq2/m2 per-i-tile so e_i unlocks as soon as tanh2_i lands
                for i in range(KT):
                    pz = psmm.tile([128, CH], f32, tag="mmtile")
                    for k in range(KT):
                        nc.tensor.matmul(
                            pz,
                            w24[:, k, i * 128 : (i + 1) * 128],
                            h1[n][:, k, :],
                            start=(k == 0),
                            stop=(k == KT - 1),
                        )
                    nc.scalar.activation(
                        h2[n][:, i, :], pz, AF.Tanh, bias=b2t[:, i : i + 1]
                    )
                    nc.vector.tensor_mul(
                        sq2[n][:, i, :], h2[n][:, i, :], h2[n][:, i, :]
                    )
                    nc.vector.tensor_scalar_sub(
                        m2[n][:, i, :], sq2[n][:, i, :], 1.0
                    )

                # -------- L3 (only needs h2; PE work while DVE does e) -----
                outT = apool.tile([D + 1, CH], f32, tag=f"outT{n}", name=f"outT{n}")
                po = psmm.tile([D, CH], f32, tag="mmtile", name=f"po{n}")
                for k in range(KT):
                    nc.tensor.matmul(
                        po,
                        w34[:, k, :],
                        h2[n][:, k, :],
                        start=(k == 0),
                        stop=(k == KT - 1),
                    )
                nc.scalar.activation(outT[0:D, :], po, AF.Identity, bias=b3t)

                # -------- c = G^T m1 ; e_i = m2_i * pc_i -------------------
                for i in range(KT):
                    pc = psmm.tile([128, CH], f32, tag="mmtile")
                    for k in range(KT):
                        nc.tensor.matmul(
                            pc,
                            g24[:, k, i * 128 : (i + 1) * 128],
                            m1[n][:, k, :],
                            start=(k == 0),
                            stop=(k == KT - 1),
                        )
                    nc.vector.tensor_mul(ebuf[n][:, i, :], m2[n][:, i, :], pc)

                # -------- div: 4 accumulating ones-MMs over e k-tiles ------
                pd = psmm.tile([1, CH], f32, tag="mmtile", name=f"pd{n}")
                for k in range(KT):
                    nc.tensor.matmul(
                        pd,
                        ones_col,
                        ebuf[n][:, k, :],
                        start=(k == 0),
                        stop=(k == KT - 1),
                    )
                nc.scalar.activation(outT[D : D + 1, :], pd, AF.Identity)

                nc.sync.dma_start(
                    out=out_f[:, n * CH : (n + 1) * CH], in_=outT
                )

    if not for_sim:
        _split_excess_waits(nc)
    return nc


def _get_nc():
    if "nc" not in _CACHE:
        _CACHE["nc"] = _build()
    return _CACHE["nc"]


def _np_ddt():
    import concourse.mybir as mybir

    if MM_DTYPE in ("float32", "float32r"):
        return np.float32
    return mybir.dt.np(getattr(mybir.dt, MM_DTYPE))


def _make_in_maps(inputs):
    t = np.asarray(inputs["t"], np.float32)
    x = np.asarray(inputs["x"], np.float32)
    W1 = np.asarray(inputs["W1"], np.float32)
    b1 = np.asarray(inputs["b1"], np.float32)
    W2 = np.asarray(inputs["W2"], np.float32)
    b2 = np.asarray(inputs["b2"], np.float32)
    W3 = np.asarray(inputs["W3"], np.float32)
    b3 = np.asarray(inputs["b3"], np.float32)
    ddt = _np_ddt()

    # feature-major xhat: rows 0..63 = x^T, row 64 = t, row 65 = 1
    xhat = np.empty((D + 2, B), np.float32)
    xhat[0:D] = x[:, 0:D].T
    xhat[D] = t[0]
    xhat[D + 1] = 1.0
    xhat = xhat.astype(ddt)

    w1hat = np.concatenate([W1[1:], W1[0:1], b1[None]], axis=0)  # [66, H]

    # host G = W2 * (W1[1:].T @ W3.T)   [H, H]
    G = (W2 * (W1[1:].T @ W3.T)).astype(np.float32)

    bias = np.zeros((128, KT + 1), np.float32)
    bias[:, 0:KT] = b2.reshape(KT, 128).T
    bias[0:D, KT] = b3

    base = {
        "w1hat_h": np.ascontiguousarray(w1hat.astype(ddt)),
        "w2_h": np.ascontiguousarray(W2.astype(ddt)),
        "g_h": np.ascontiguousarray(G.astype(ddt)),
        "w3_h": np.ascontiguousarray(W3.astype(ddt)),
        "bias_h": bias,
        "ones_h": np.ones((128, 1), ddt),
    }
    return [
        dict(base, xhat_h=np.ascontiguousarray(xhat[:, i * BC : (i + 1) * BC]))
        for i in range(N_CORES)
    ]


def _gather(res):
    # device output is feature-major [65, BC]; transpose back per core
    return np.concatenate(
        [np.ascontiguousarray(res.results[i]["out_f"].T) for i in range(N_CORES)],
        axis=0,
    )


def kernel(t, x, W1, b1, W2, b2, W3, b3):
    from concourse.bass_utils import run_bass_kernel_spmd

    nc = _get_nc()
    in_maps = _make_in_maps(
        dict(t=t, x=x, W1=W1, b1=b1, W2=W2, b2=b2, W3=W3, b3=b3)
    )
    res = run_bass_kernel_spmd(nc, in_maps, core_ids=list(range(N_CORES)))
    _CACHE["last_result"] = res
    out = _gather(res)
    # flaky-core guard: a dropped execution leaves the donated zero output
    # buffer untouched; the true output of this MLP is never all-zero.
    for _ in range(3):
        bad = [
            i
            for i in range(N_CORES)
            if not np.any(res.results[i]["out_f"][0:D, :])
        ]
        if not bad:
            break
        res = run_bass_kernel_spmd(nc, in_maps, core_ids=list(range(N_CORES)))
        _CACHE["last_result"] = res
        out = _gather(res)
    return out
